# revision 24
# baseline (speedup 1.0000x reference)
"""Single-head causal attention (B=4, T=2048, C=1024, H=64) on 8 NeuronCores.

Sharding: 8 cores = 4 batches x 2 interleaved halves. Core (b, h) computes
query blocks of 512 rows: blk0 = rows [h*512, h*512+512), blk1 = rows
[1024+h*512, 1024+h*512+512).

Design notes:
  - ONE x layout of 4 slots of 512 rows per core: slot0 = blk0 query rows,
    slot1 = blk1 query rows, slot2/slot3 = the remaining prefix rows (per-core
    DATA chooses which; h=0's slot3 is dead padding killed by the bias rows).
    The k/v projection of the slots serves BOTH the full-phase keys and the
    diagonal keys (no separate diag-key projection, no duplicated x DMA).
  - PE p-states: TRN2's PE runs at ~1.2GHz until ~6us of continuous
    execution, then 2.4GHz. A warmup chain of matmuls on a zeroed tile runs
    during the DMA lead-in (memset is the FIRST gpsimd instruction so warmup
    can start right after engine init), and the PE program order afterwards is
    arranged to have no long waits.
  - DMA: x slots are split in half across the sync+scalar HWDGE queues in
    arrival-priority order xs0, xs1, xs2, xs3 (weights first); constants ride
    the gpsimd SWDGE queue. Score groups are emitted in DMA-arrival order so
    the scalar engine's exp() chain (the only engine with activation, ~1.1us
    per [128,1024] group) never starves: b0{s0 diag}, b1{s0 full}, b1{s1
    diag}, b0{s2}, b1{s2}, b1{s3}.
  - Causality is data-driven: kt rows 64:66 hold per-(block, slot) biases
    (0 or -1e30/scale), qb rows 64:66 hold block selectors, diagonal chunks
    are masked post-exp with slices of a shared staircase tile.
  - Scores are computed transposed (scoresT[tk, tq]); V is augmented with a
    ones column so PV psum row 64 is the softmax denominator. Both blocks'
    PV accumulators are live concurrently (2 psum banks) so the PE can
    interleave block1 PVs early; the proj psum tag is shared with the
    v-transpose and epilogue-transpose tiles to fit PSUM in 8 banks.
  - Epilogue: transpose 128-col slabs on PE, one batched reciprocal on DVE,
    per-slab scale on the scalar engine (idle at the tail), one DMA per block.
"""

import numpy as np
import ml_dtypes

import concourse.bass as bass
from concourse import bacc
import concourse.mybir as mybir
import concourse.tile as tile
from concourse.bass_utils import run_bass_kernel_spmd

B, T, C, H = 4, 2048, 1024, 64
P = 128
TQ = 512                 # rows per slot / query block width
NSLOT = 4
CCH = C // P             # 8 contraction chunks
NDIAG = TQ // P          # 4 chunks of 128 per slot
SCALE = float(C) ** -0.5
BIGNEG = -1e30 / SCALE   # lands as -1e30 after the exp scale
NWARM = 11               # PE warmup matmuls (p-state ramp during DMA lead-in)

F32 = mybir.dt.float32
BF16 = mybir.dt.bfloat16
NPBF = ml_dtypes.bfloat16

_CACHE = {}

# chunk schedule: per block, list of (slot, c, kind). Uniform across cores.
CHUNKS = {
    0: [(0, c, "d") for c in range(NDIAG)] + [(2, c, "f") for c in range(NDIAG)],
    1: [(0, c, "f") for c in range(NDIAG)] + [(1, c, "d") for c in range(NDIAG)]
       + [(2, c, "f") for c in range(NDIAG)] + [(3, c, "f") for c in range(NDIAG)],
}


def build():
    nc = bacc.Bacc()
    xs_d = nc.declare_dram_parameter("xs", [NSLOT, P, CCH * TQ], BF16, isOutput=False)
    wkv_d = nc.declare_dram_parameter("wkv", [P, CCH * 2 * H], BF16, isOutput=False)
    wq_d = nc.declare_dram_parameter("wq", [P, CCH * H], BF16, isOutput=False)
    kaug_d = nc.declare_dram_parameter("kaug", [2, NSLOT * TQ], BF16, isOutput=False)
    qsel_d = nc.declare_dram_parameter("qsel", [2, 2 * TQ], BF16, isOutput=False)
    st_d = nc.declare_dram_parameter("stair", [P, 896], BF16, isOutput=False)
    idb_d = nc.declare_dram_parameter("identb", [P, P], BF16, isOutput=False)
    idf_d = nc.declare_dram_parameter("identf", [H + 1, H + 1], F32, isOutput=False)
    on_d = nc.declare_dram_parameter("vones", [P, NSLOT * NDIAG], BF16, isOutput=False)
    out_d = nc.declare_dram_parameter("out", [P, 2 * NDIAG * H], F32, isOutput=True)

    EXPF = mybir.ActivationFunctionType.Exp
    COPYF = mybir.ActivationFunctionType.Copy
    KW = NSLOT * TQ  # 2048 key columns

    with tile.TileContext(nc) as tc:
        with (
            tc.tile_pool(name="big", bufs=1) as big,
            tc.tile_pool(name="work", bufs=4) as work,
            tc.tile_pool(name="pw", bufs=2, space="PSUM") as pw,
            tc.tile_pool(name="pss", bufs=2, space="PSUM") as pss,
            tc.tile_pool(name="ps_pv", bufs=2, space="PSUM") as pspv,
        ):
            # ---- warmup tile memset: FIRST vector instruction ----
            zw = big.tile([P, TQ], BF16)
            nc.vector.memset(zw[:], 0)

            # ---- DMA issues. The HW queue appears to complete in-flight ops
            # near-concurrently (round-robin-ish), so the FIRST-needed pieces
            # are kept SMALL: weights split in half, slot0 in 128KB quarters;
            # later slots ride as 512KB halves. wq/wkvB on scalar (lighter),
            # wkvA on sync ----
            wkv = big.tile([P, CCH, 2 * H], BF16)
            wq = big.tile([P, CCH, H], BF16)
            xts = []
            for s in range(NSLOT):
                xts.append(big.tile([P, CCH, TQ], BF16, tag=f"xs{s}", name=f"xs{s}"))
            HC = CCH // 2
            QC = CCH // 4
            nc.sync.dma_start(out=wkv[:, 0:HC, :],
                              in_=wkv_d[:, 0:HC * 2 * H].rearrange("p (nc h) -> p nc h", nc=HC))
            nc.scalar.dma_start(out=wq[:],
                                in_=wq_d[:].rearrange("p (nc h) -> p nc h", nc=CCH))
            nc.scalar.dma_start(out=wkv[:, HC:CCH, :],
                                in_=wkv_d[:, HC * 2 * H:].rearrange("p (nc h) -> p nc h", nc=HC))
            # slot0: 4 quarters (cc01/cc23 on sync, cc45/cc67 on scalar)
            for qi, eng in ((0, nc.sync), (2, nc.scalar), (1, nc.sync), (3, nc.scalar)):
                eng.dma_start(
                    out=xts[0][:, qi * QC:(qi + 1) * QC, :],
                    in_=xs_d[0, :, qi * QC * TQ:(qi + 1) * QC * TQ]
                    .rearrange("p (nc t) -> p nc t", nc=QC))
            for s in (1, 2, 3):      # later slots: halves
                t = xts[s]
                nc.sync.dma_start(
                    out=t[:, 0:HC, :],
                    in_=xs_d[s, :, 0:HC * TQ].rearrange("p (nc t) -> p nc t", nc=HC))
                nc.scalar.dma_start(
                    out=t[:, HC:CCH, :],
                    in_=xs_d[s, :, HC * TQ:].rearrange("p (nc t) -> p nc t", nc=HC))
            # constants on the gpsimd SWDGE queue (after the zw memset)
            kt = big.tile([66, KW], BF16)
            nc.gpsimd.dma_start(out=kt[64:66, :], in_=kaug_d[:])
            qb = big.tile([66, 2 * TQ], BF16)
            nc.gpsimd.dma_start(out=qb[64:66, :], in_=qsel_d[:])
            identb = big.tile([P, P], BF16)
            nc.gpsimd.dma_start(out=identb[:], in_=idb_d[:])
            stair = big.tile([P, 896], BF16)
            nc.gpsimd.dma_start(out=stair[:], in_=st_d[:])
            vtmp = big.tile([P, NSLOT * NDIAG], BF16)
            nc.gpsimd.dma_start(out=vtmp[:], in_=on_d[:])
            identf = big.tile([H + 1, H + 1], F32)
            nc.gpsimd.dma_start(out=identf[:], in_=idf_d[:])

            # ---- persistent sbuf tiles ----
            vh = big.tile([P, KW], BF16)            # rows 64:128 = vT
            vaug = big.tile([P, NSLOT * NDIAG, H + 1], BF16)
            nc.vector.tensor_copy(vaug[:, :, H], vtmp[:])

            # ---- PE warmup (p-state ramp while DMA streams) ----
            wps = pss.tile([P, TQ], F32, tag="s")
            for _ in range(NWARM):
                nc.tensor.matmul(wps[:], zw[:, 0:P], zw[:], start=True, stop=True)

            # ---- emission helpers ----
            # cc order matched to piece arrival order
            CCORD = (0, 1, 4, 5, 2, 3, 6, 7)

            def emit_kv(s):
                ps = pw.tile([P, TQ], F32, tag="proj", name=f"pkv{s}")
                for cc in CCORD:
                    nc.tensor.matmul(ps[:], wkv[:, cc, :], xts[s][:, cc, :],
                                     start=(cc == CCORD[0]), stop=(cc == CCORD[-1]))
                nc.vector.tensor_copy(kt[0:64, bass.ts(s, TQ)], ps[0:64, :])
                nc.vector.tensor_copy(vh[64:128, bass.ts(s, TQ)], ps[64:128, :])

            def emit_q(blk):
                ps = pw.tile([64, TQ], F32, tag="proj", name=f"pq{blk}")
                for cc in CCORD:
                    nc.tensor.matmul(ps[:], wq[:, cc, :], xts[blk][:, cc, :],
                                     start=(cc == CCORD[0]), stop=(cc == CCORD[-1]))
                nc.vector.tensor_copy(qb[0:64, bass.ts(blk, TQ)], ps[:])

            def emit_T(s):
                tp = pw.tile([P, NDIAG, H], BF16, tag="proj", name=f"tp{s}")
                for c in range(NDIAG):
                    nc.tensor.transpose(tp[:, c, :],
                                        vh[64:128, s * TQ + c * P: s * TQ + (c + 1) * P],
                                        identb[64:128, 64:128])
                nc.vector.tensor_copy(vaug[:, s * NDIAG:(s + 1) * NDIAG, 0:H], tp[:])

            e_tiles = {}

            def emit_sc(blk, g):
                s = pss.tile([P, 2 * TQ], F32, tag="s", name=f"s{blk}_{g}")
                for gi in range(2):
                    slot, c, _ = CHUNKS[blk][2 * g + gi]
                    nc.tensor.matmul(s[:, bass.ts(gi, TQ)],
                                     kt[:, slot * TQ + c * P: slot * TQ + (c + 1) * P],
                                     qb[:, bass.ts(blk, TQ)], start=True, stop=True)
                e = work.tile([P, 2 * TQ], BF16, tag="e", name=f"e{blk}_{g}")
                nc.scalar.activation(e[:], s[:], EXPF, scale=SCALE)
                e_tiles[(blk, g)] = e

            pv_tiles = {}

            def emit_pv(blk, g):
                if g == 0:
                    pv_tiles[blk] = pspv.tile([H + 1, TQ], F32, tag="pv",
                                              name=f"pv{blk}")
                pv = pv_tiles[blk]
                e = e_tiles.pop((blk, g))
                # stair masks emitted just-in-time (keeps earlier vector
                # copies, which the act chain depends on, unblocked)
                for gi in range(2):
                    slot, c, kind = CHUNKS[blk][2 * g + gi]
                    if kind == "d":
                        off = 384 - P * c
                        nc.vector.tensor_mul(e[:, bass.ts(gi, TQ)],
                                             e[:, bass.ts(gi, TQ)],
                                             stair[:, off:off + TQ])
                n = len(CHUNKS[blk])
                for gi in range(2):
                    idx = 2 * g + gi
                    slot, c, _ = CHUNKS[blk][idx]
                    nc.tensor.matmul(pv[:], vaug[:, slot * NDIAG + c, :],
                                     e[:, bass.ts(gi, TQ)],
                                     start=(idx == 0), stop=(idx == n - 1))

            pvs_tiles = {}

            def emit_epi_copy(blk):
                # psum -> sbuf; must precede any reuse of the pv psum buffer.
                # Split in two so the first transposes can start early.
                pvs = work.tile([H + 1, TQ], F32, tag="pvs", name=f"pvs{blk}")
                nc.vector.tensor_copy(pvs[:, 0:TQ // 2], pv_tiles[blk][:, 0:TQ // 2])
                nc.vector.tensor_copy(pvs[:, TQ // 2:], pv_tiles[blk][:, TQ // 2:])
                pvs_tiles[blk] = pvs

            def emit_epi_rest(blk):
                pvs = pvs_tiles.pop(blk)
                ob = work.tile([P, NDIAG, H], F32, tag="ob", name=f"ob{blk}")
                ot = pw.tile([P, NDIAG, H + 1], F32, tag="proj", name=f"ot{blk}")
                for j in range(NDIAG):
                    nc.tensor.transpose(ot[:, j, :], pvs[:, bass.ts(j, P)], identf[:])
                r4 = work.tile([P, NDIAG], F32, tag="r", name=f"r{blk}")
                nc.vector.reciprocal(r4[:], ot[:, :, H])
                for j in range(NDIAG):
                    # out = ot * (1/denom); alternate scalar/vector engines
                    if j % 2 == 0:
                        nc.scalar.activation(ob[:, j, :], ot[:, j, 0:H], COPYF,
                                             scale=r4[:, j:j + 1])
                    else:
                        nc.vector.tensor_scalar_mul(ob[:, j, :], ot[:, j, 0:H],
                                                    r4[:, j:j + 1])
                nc.sync.dma_start(out=out_d[:, blk * NDIAG * H:(blk + 1) * NDIAG * H],
                                  in_=ob[:])

            # ---- schedule (per-engine program order == emission order).
            # Score groups are emitted in DMA-arrival order so the scalar
            # exp() chain never starves; projections/transposes fill the PE
            # while activations run.
            emit_kv(0)
            emit_q(0)
            emit_sc(0, 0)       # s0 diag
            emit_sc(0, 1)       # s0 diag
            emit_T(0)
            emit_kv(1)
            emit_q(1)
            emit_sc(1, 0)       # s0 full
            emit_sc(1, 1)       # s0 full
            emit_T(1)
            emit_pv(0, 0)
            emit_pv(0, 1)
            emit_sc(1, 2)       # s1 diag
            emit_sc(1, 3)       # s1 diag
            emit_kv(2)
            emit_pv(1, 0)
            emit_pv(1, 1)
            emit_T(2)
            emit_sc(0, 2)       # s2 full
            emit_sc(0, 3)       # s2 full
            emit_kv(3)
            emit_pv(1, 2)
            emit_pv(1, 3)
            emit_sc(1, 4)       # s2 full
            emit_sc(1, 5)       # s2 full
            emit_T(3)
            emit_pv(0, 2)
            emit_pv(0, 3)       # blk0 STOP
            emit_epi_copy(0)
            emit_sc(1, 6)       # s3 full
            emit_sc(1, 7)       # s3 full
            emit_pv(1, 4)
            emit_pv(1, 5)
            emit_epi_rest(0)
            emit_pv(1, 6)
            emit_pv(1, 7)       # blk1 STOP
            emit_epi_copy(1)
            emit_epi_rest(1)
    nc.compile()
    return nc


def _pack_x(xT, cols):
    # xT: [C, T] fp32 -> [P, CCH*W] bf16 in SBUF layout
    a = xT[:, cols]                                   # [C, W]
    a = a.reshape(CCH, P, -1).transpose(1, 0, 2)      # [P, CCH, W]
    return np.ascontiguousarray(a.reshape(P, -1)).astype(NPBF)


def _pack_w(w):
    # w: [C, width] -> [P, CCH*width]
    a = w.reshape(CCH, P, -1).transpose(1, 0, 2)
    return np.ascontiguousarray(a.reshape(P, -1)).astype(NPBF)


def _host_inputs(x, Wk, Wq, Wv):
    wkv = _pack_w(np.concatenate([Wk, Wv], axis=1))
    wq = _pack_w(Wq)
    ii = np.arange(P)
    stair = (np.arange(896)[None, :] >= ii[:, None] + 384).astype(NPBF)
    identb = np.eye(P, dtype=NPBF)
    identf = np.eye(H + 1, dtype=np.float32)
    vones = np.ones((P, NSLOT * NDIAG), NPBF)
    qsel = np.zeros((2, 2 * TQ), np.float32)
    qsel[0, :TQ] = 1.0
    qsel[1, TQ:] = 1.0
    qsel = qsel.astype(NPBF)
    in_maps = []
    for b in range(B):
        xT = np.ascontiguousarray(x[b].T.astype(np.float32))  # [C, T]
        for h in range(2):
            if h == 0:
                rows = [(0, 512), (1024, 1536), (512, 1024), (512, 1024)]
            else:
                rows = [(512, 1024), (1536, 2048), (0, 512), (1024, 1536)]
            xs = np.stack([_pack_x(xT, slice(a, bb)) for (a, bb) in rows])
            q0s = (h * TQ, 1024 + h * TQ)
            # bias rows: kaug[blk, slot cols] = 0 if slot rows fully causal
            # for that block (or the block's own diag slot), else BIGNEG
            kaug = np.full((2, NSLOT * TQ), BIGNEG, np.float32)
            for blk in range(2):
                for s, (a, bb) in enumerate(rows):
                    if s == blk:
                        kaug[blk, s * TQ:(s + 1) * TQ] = 0.0   # diag slot
                    elif bb <= q0s[blk] and not (h == 0 and s == 3):
                        kaug[blk, s * TQ:(s + 1) * TQ] = 0.0   # fully causal
            in_maps.append(dict(xs=xs, wkv=wkv, wq=wq, kaug=kaug.astype(NPBF),
                                qsel=qsel, stair=stair, identb=identb,
                                identf=identf, vones=vones))
    return in_maps


def kernel(x, Wk, Wq, Wv, trace=False):
    x = np.asarray(x, np.float32)
    in_maps = _host_inputs(x, np.asarray(Wk, np.float32),
                           np.asarray(Wq, np.float32), np.asarray(Wv, np.float32))
    if "nc" not in _CACHE:
        _CACHE["nc"] = build()
    nc = _CACHE["nc"]
    res = run_bass_kernel_spmd(nc, in_maps, list(range(8)), trace=trace)
    out = np.empty((B, T, H), np.float32)
    for b in range(B):
        for h in range(2):
            o = res.results[b * 2 + h]["out"]  # [P, 2*NDIAG*H]
            o = np.asarray(o).reshape(P, 2, NDIAG, H)
            q0s = (h * TQ, 1024 + h * TQ)
            for blk, q0 in enumerate(q0s):
                # row q0 + j*128 + p  <-  o[p, blk, j, :]
                out[b, q0:q0 + TQ] = o[:, blk].transpose(1, 0, 2).reshape(TQ, H)
    kernel.last_exec_time_ns = res.exec_time_ns
    kernel.last_results = res
    return out


# revision 30
# speedup vs baseline: 1.1024x; 1.1024x over previous
"""Single-head causal attention (B=4, T=2048, C=1024, H=64) on 8 NeuronCores.

Sharding: 8 cores = 4 batches x 2 interleaved halves. Core (b, h) computes
query blocks of 512 rows: blk0 = rows [h*512, h*512+512), blk1 = rows
[1024+h*512, 1024+h*512+512).

Design notes:
  - ONE x layout of 4 slots of 512 rows per core: slot0 = blk0 query rows,
    slot1 = blk1 query rows, slot2/slot3 = the remaining prefix rows (per-core
    DATA chooses which; h=0's slot3 is dead padding killed by the bias rows).
    The k/v projection of the slots serves BOTH the full-phase keys and the
    diagonal keys (no separate diag-key projection, no duplicated x DMA).
  - PE p-states: TRN2's PE runs at ~1.2GHz until ~6us of continuous
    execution, then 2.4GHz. A warmup chain of matmuls on a zeroed tile runs
    during the DMA lead-in (memset is the FIRST gpsimd instruction so warmup
    can start right after engine init), and the PE program order afterwards is
    arranged to have no long waits.
  - DMA: x slots are split in half across the sync+scalar HWDGE queues in
    arrival-priority order xs0, xs1, xs2, xs3 (weights first); constants ride
    the gpsimd SWDGE queue. Score groups are emitted in DMA-arrival order so
    the scalar engine's exp() chain (the only engine with activation, ~1.1us
    per [128,1024] group) never starves: b0{s0 diag}, b1{s0 full}, b1{s1
    diag}, b0{s2}, b1{s2}, b1{s3}.
  - Causality is data-driven: kt rows 64:66 hold per-(block, slot) biases
    (0 or -1e30/scale), qb rows 64:66 hold block selectors, diagonal chunks
    are masked post-exp with slices of a shared staircase tile.
  - Scores are computed transposed (scoresT[tk, tq]); V is augmented with a
    ones column so PV psum row 64 is the softmax denominator. Both blocks'
    PV accumulators are live concurrently (2 psum banks) so the PE can
    interleave block1 PVs early; the proj psum tag is shared with the
    v-transpose and epilogue-transpose tiles to fit PSUM in 8 banks.
  - Epilogue: transpose 128-col slabs on PE, one batched reciprocal on DVE,
    per-slab scale on the scalar engine (idle at the tail), one DMA per block.
"""

import numpy as np
import ml_dtypes

import concourse.bass as bass
from concourse import bacc
import concourse.mybir as mybir
import concourse.tile as tile
from concourse.bass_utils import run_bass_kernel_spmd

B, T, C, H = 4, 2048, 1024, 64
P = 128
TQ = 512                 # rows per slot / query block width
NSLOT = 4
CCH = C // P             # 8 contraction chunks
NDIAG = TQ // P          # 4 chunks of 128 per slot
SCALE = float(C) ** -0.5
BIGNEG = -1e30 / SCALE   # lands as -1e30 after the exp scale
NWARM = 10               # PE warmup matmuls (p-state ramp during DMA lead-in)

F32 = mybir.dt.float32
BF16 = mybir.dt.bfloat16
NPBF = ml_dtypes.bfloat16

_CACHE = {}

# chunk schedule: per block, list of (slot, c, kind). Uniform across cores.
CHUNKS = {
    0: [(0, c, "d") for c in range(NDIAG)] + [(2, c, "f") for c in range(NDIAG)],
    1: [(0, c, "f") for c in range(NDIAG)] + [(1, c, "d") for c in range(NDIAG)]
       + [(2, c, "f") for c in range(NDIAG)] + [(3, c, "f") for c in range(NDIAG)],
}


def build():
    nc = bacc.Bacc()
    xs_d = nc.declare_dram_parameter("xs", [NSLOT, P, CCH * TQ], BF16, isOutput=False)
    wkv_d = nc.declare_dram_parameter("wkv", [P, CCH * 2 * H], BF16, isOutput=False)
    wq_d = nc.declare_dram_parameter("wq", [P, CCH * H], BF16, isOutput=False)
    kaug_d = nc.declare_dram_parameter("kaug", [2, NSLOT * TQ], BF16, isOutput=False)
    qsel_d = nc.declare_dram_parameter("qsel", [2, 2 * TQ], BF16, isOutput=False)
    st_d = nc.declare_dram_parameter("stair", [P, 896], BF16, isOutput=False)
    idb_d = nc.declare_dram_parameter("identb", [P, P], BF16, isOutput=False)
    on_d = nc.declare_dram_parameter("vones", [P, NSLOT * NDIAG], BF16, isOutput=False)
    out_d = nc.declare_dram_parameter("out", [P, 2 * NDIAG * H], F32, isOutput=True)

    EXPF = mybir.ActivationFunctionType.Exp
    COPYF = mybir.ActivationFunctionType.Copy
    KW = NSLOT * TQ  # 2048 key columns

    with tile.TileContext(nc) as tc:
        with (
            tc.tile_pool(name="big", bufs=1) as big,
            tc.tile_pool(name="work", bufs=4) as work,
            tc.tile_pool(name="pw", bufs=2, space="PSUM") as pw,
            tc.tile_pool(name="pss", bufs=2, space="PSUM") as pss,
            tc.tile_pool(name="ps_pv", bufs=2, space="PSUM") as pspv,
        ):
            # ---- warmup tile memset: FIRST vector instruction ----
            zw = big.tile([P, TQ], BF16)
            nc.vector.memset(zw[:], 0)

            # ---- DMA issues. Each DMA op has multi-us completion latency, so
            # few, big ops win: weights first (small), then one half-slot op
            # per queue in slot-priority order 0,1,2,3. sync carries cc0-3,
            # scalar carries cc4-7 ----
            wkv = big.tile([P, CCH, 2 * H], BF16)
            wq = big.tile([P, CCH, H], BF16)
            xts = []
            for s in range(NSLOT):
                xts.append(big.tile([P, CCH, TQ], BF16, tag=f"xs{s}", name=f"xs{s}"))
            HC = CCH // 2
            nc.sync.dma_start(out=wkv[:],
                              in_=wkv_d[:].rearrange("p (nc h) -> p nc h", nc=CCH))
            nc.scalar.dma_start(out=wq[:],
                                in_=wq_d[:].rearrange("p (nc h) -> p nc h", nc=CCH))
            for s in (0, 1, 2, 3):
                t = xts[s]
                nc.sync.dma_start(
                    out=t[:, 0:HC, :],
                    in_=xs_d[s, :, 0:HC * TQ].rearrange("p (nc t) -> p nc t", nc=HC))
                nc.scalar.dma_start(
                    out=t[:, HC:CCH, :],
                    in_=xs_d[s, :, HC * TQ:].rearrange("p (nc t) -> p nc t", nc=HC))
            # constants on the gpsimd SWDGE queue (after the zw memset)
            kt = big.tile([66, KW], BF16)
            nc.gpsimd.dma_start(out=kt[64:66, :], in_=kaug_d[:])
            qb = big.tile([66, 2 * TQ], BF16)
            nc.gpsimd.dma_start(out=qb[64:66, :], in_=qsel_d[:])
            identb = big.tile([P, P], BF16)
            nc.gpsimd.dma_start(out=identb[:], in_=idb_d[:])
            stair = big.tile([P, 896], BF16)
            nc.gpsimd.dma_start(out=stair[:], in_=st_d[:])
            vtmp = big.tile([P, NSLOT * NDIAG], BF16)
            nc.gpsimd.dma_start(out=vtmp[:], in_=on_d[:])

            # ---- persistent sbuf tiles ----
            vh = big.tile([P, KW], BF16)            # rows 64:128 = vT
            vaug = big.tile([P, NSLOT * NDIAG, H + 1], BF16)
            nc.vector.tensor_copy(vaug[:, :, H], vtmp[:])

            # ---- PE warmup (p-state ramp while DMA streams) ----
            wps = pss.tile([P, TQ], F32, tag="s")
            for _ in range(NWARM):
                nc.tensor.matmul(wps[:], zw[:, 0:P], zw[:], start=True, stop=True)

            # ---- emission helpers ----
            # cc order matched to piece arrival order (sync half, scalar half)
            CCORD = (0, 1, 2, 3, 4, 5, 6, 7)

            def emit_kv(s):
                ps = pw.tile([P, TQ], F32, tag="proj", name=f"pkv{s}")
                for cc in CCORD:
                    nc.tensor.matmul(ps[:], wkv[:, cc, :], xts[s][:, cc, :],
                                     start=(cc == CCORD[0]), stop=(cc == CCORD[-1]))
                nc.vector.tensor_copy(kt[0:64, bass.ts(s, TQ)], ps[0:64, :])
                nc.vector.tensor_copy(vh[64:128, bass.ts(s, TQ)], ps[64:128, :])

            def emit_q(blk):
                ps = pw.tile([64, TQ], F32, tag="proj", name=f"pq{blk}")
                for cc in CCORD:
                    nc.tensor.matmul(ps[:], wq[:, cc, :], xts[blk][:, cc, :],
                                     start=(cc == CCORD[0]), stop=(cc == CCORD[-1]))
                nc.vector.tensor_copy(qb[0:64, bass.ts(blk, TQ)], ps[:])

            def emit_T(s):
                tp = pw.tile([P, NDIAG, H], BF16, tag="proj", name=f"tp{s}")
                for c in range(NDIAG):
                    nc.tensor.transpose(tp[:, c, :],
                                        vh[64:128, s * TQ + c * P: s * TQ + (c + 1) * P],
                                        identb[64:128, 64:128])
                nc.vector.tensor_copy(vaug[:, s * NDIAG:(s + 1) * NDIAG, 0:H], tp[:])

            e_tiles = {}

            def emit_sc(blk, g):
                s = pss.tile([P, 2 * TQ], F32, tag="s", name=f"s{blk}_{g}")
                for gi in range(2):
                    slot, c, _ = CHUNKS[blk][2 * g + gi]
                    nc.tensor.matmul(s[:, bass.ts(gi, TQ)],
                                     kt[:, slot * TQ + c * P: slot * TQ + (c + 1) * P],
                                     qb[:, bass.ts(blk, TQ)], start=True, stop=True)
                e = work.tile([P, 2 * TQ], BF16, tag="e", name=f"e{blk}_{g}")
                nc.scalar.activation(e[:], s[:], EXPF, scale=SCALE)
                e_tiles[(blk, g)] = e

            pv_tiles = {}

            def emit_pv(blk, g):
                if g == 0:
                    pv_tiles[blk] = pspv.tile([H + 1, TQ], F32, tag="pv",
                                              name=f"pv{blk}")
                pv = pv_tiles[blk]
                e = e_tiles.pop((blk, g))
                # stair masks emitted just-in-time (keeps earlier vector
                # copies, which the act chain depends on, unblocked)
                for gi in range(2):
                    slot, c, kind = CHUNKS[blk][2 * g + gi]
                    if kind == "d":
                        off = 384 - P * c
                        nc.vector.tensor_mul(e[:, bass.ts(gi, TQ)],
                                             e[:, bass.ts(gi, TQ)],
                                             stair[:, off:off + TQ])
                n = len(CHUNKS[blk])
                for gi in range(2):
                    idx = 2 * g + gi
                    slot, c, _ = CHUNKS[blk][idx]
                    nc.tensor.matmul(pv[:], vaug[:, slot * NDIAG + c, :],
                                     e[:, bass.ts(gi, TQ)],
                                     start=(idx == 0), stop=(idx == n - 1))

            pvs_tiles = {}

            def emit_epi_copy(blk):
                # psum -> sbuf (bf16: halves transpose cost; precision is
                # ample); must precede any reuse of the pv psum buffer.
                # Split in two so the first transposes can start early.
                pvs = work.tile([H + 1, TQ], BF16, tag="pvs", name=f"pvs{blk}")
                nc.vector.tensor_copy(pvs[:, 0:TQ // 2], pv_tiles[blk][:, 0:TQ // 2])
                nc.vector.tensor_copy(pvs[:, TQ // 2:], pv_tiles[blk][:, TQ // 2:])
                pvs_tiles[blk] = pvs

            def emit_epi_rest(blk):
                pvs = pvs_tiles.pop(blk)
                ob = work.tile([P, NDIAG, H], F32, tag="ob", name=f"ob{blk}")
                # H+2 columns: keeps each slab's PSUM offset 4-byte aligned
                ot = pw.tile([P, NDIAG, H + 2], BF16, tag="proj", name=f"ot{blk}")
                for j in range(NDIAG):
                    nc.tensor.transpose(ot[:, j, 0:H + 1], pvs[:, bass.ts(j, P)],
                                        identb[0:H + 1, 0:H + 1])
                r4 = work.tile([P, NDIAG], F32, tag="r", name=f"r{blk}")
                nc.vector.reciprocal(r4[:], ot[:, :, H])
                for j in range(NDIAG):
                    # out = ot * (1/denom); alternate scalar/vector engines
                    if j % 2 == 0:
                        nc.scalar.activation(ob[:, j, :], ot[:, j, 0:H], COPYF,
                                             scale=r4[:, j:j + 1])
                    else:
                        nc.vector.tensor_scalar_mul(ob[:, j, :], ot[:, j, 0:H],
                                                    r4[:, j:j + 1])
                nc.sync.dma_start(out=out_d[:, blk * NDIAG * H:(blk + 1) * NDIAG * H],
                                  in_=ob[:])

            # ---- schedule (per-engine program order == emission order).
            # Score groups are emitted in DMA-arrival order so the scalar
            # exp() chain never starves; projections/transposes fill the PE
            # while activations run.
            emit_kv(0)
            emit_q(0)
            emit_sc(0, 0)       # s0 diag
            emit_sc(0, 1)       # s0 diag
            emit_T(0)
            emit_kv(1)
            emit_q(1)
            emit_sc(1, 0)       # s0 full
            emit_sc(1, 1)       # s0 full
            emit_T(1)
            emit_pv(0, 0)
            emit_pv(0, 1)
            emit_sc(1, 2)       # s1 diag
            emit_sc(1, 3)       # s1 diag
            emit_kv(2)
            emit_pv(1, 0)
            emit_pv(1, 1)
            emit_T(2)
            emit_sc(0, 2)       # s2 full
            emit_sc(0, 3)       # s2 full
            emit_kv(3)
            emit_pv(1, 2)
            emit_pv(1, 3)
            emit_sc(1, 4)       # s2 full
            emit_sc(1, 5)       # s2 full
            emit_T(3)
            emit_pv(0, 2)
            emit_pv(0, 3)       # blk0 STOP
            emit_epi_copy(0)
            emit_sc(1, 6)       # s3 full
            emit_sc(1, 7)       # s3 full
            emit_pv(1, 4)
            emit_pv(1, 5)
            emit_epi_rest(0)
            emit_pv(1, 6)
            emit_pv(1, 7)       # blk1 STOP
            emit_epi_copy(1)
            emit_epi_rest(1)
    nc.compile()
    return nc


def _pack_x(xT, cols):
    # xT: [C, T] fp32 -> [P, CCH*W] bf16 in SBUF layout
    a = xT[:, cols]                                   # [C, W]
    a = a.reshape(CCH, P, -1).transpose(1, 0, 2)      # [P, CCH, W]
    return np.ascontiguousarray(a.reshape(P, -1)).astype(NPBF)


def _pack_w(w):
    # w: [C, width] -> [P, CCH*width]
    a = w.reshape(CCH, P, -1).transpose(1, 0, 2)
    return np.ascontiguousarray(a.reshape(P, -1)).astype(NPBF)


def _host_inputs(x, Wk, Wq, Wv):
    wkv = _pack_w(np.concatenate([Wk, Wv], axis=1))
    wq = _pack_w(Wq)
    ii = np.arange(P)
    stair = (np.arange(896)[None, :] >= ii[:, None] + 384).astype(NPBF)
    identb = np.eye(P, dtype=NPBF)
    vones = np.ones((P, NSLOT * NDIAG), NPBF)
    qsel = np.zeros((2, 2 * TQ), np.float32)
    qsel[0, :TQ] = 1.0
    qsel[1, TQ:] = 1.0
    qsel = qsel.astype(NPBF)
    in_maps = []
    for b in range(B):
        xT = np.ascontiguousarray(x[b].T.astype(np.float32))  # [C, T]
        for h in range(2):
            if h == 0:
                rows = [(0, 512), (1024, 1536), (512, 1024), (512, 1024)]
            else:
                rows = [(512, 1024), (1536, 2048), (0, 512), (1024, 1536)]
            xs = np.stack([_pack_x(xT, slice(a, bb)) for (a, bb) in rows])
            q0s = (h * TQ, 1024 + h * TQ)
            # bias rows: kaug[blk, slot cols] = 0 if slot rows fully causal
            # for that block (or the block's own diag slot), else BIGNEG
            kaug = np.full((2, NSLOT * TQ), BIGNEG, np.float32)
            for blk in range(2):
                for s, (a, bb) in enumerate(rows):
                    if s == blk:
                        kaug[blk, s * TQ:(s + 1) * TQ] = 0.0   # diag slot
                    elif bb <= q0s[blk] and not (h == 0 and s == 3):
                        kaug[blk, s * TQ:(s + 1) * TQ] = 0.0   # fully causal
            in_maps.append(dict(xs=xs, wkv=wkv, wq=wq, kaug=kaug.astype(NPBF),
                                qsel=qsel, stair=stair, identb=identb,
                                vones=vones))
    return in_maps


def kernel(x, Wk, Wq, Wv, trace=False):
    x = np.asarray(x, np.float32)
    in_maps = _host_inputs(x, np.asarray(Wk, np.float32),
                           np.asarray(Wq, np.float32), np.asarray(Wv, np.float32))
    if "nc" not in _CACHE:
        _CACHE["nc"] = build()
    nc = _CACHE["nc"]
    res = run_bass_kernel_spmd(nc, in_maps, list(range(8)), trace=trace)
    out = np.empty((B, T, H), np.float32)
    for b in range(B):
        for h in range(2):
            o = res.results[b * 2 + h]["out"]  # [P, 2*NDIAG*H]
            o = np.asarray(o).reshape(P, 2, NDIAG, H)
            q0s = (h * TQ, 1024 + h * TQ)
            for blk, q0 in enumerate(q0s):
                # row q0 + j*128 + p  <-  o[p, blk, j, :]
                out[b, q0:q0 + TQ] = o[:, blk].transpose(1, 0, 2).reshape(TQ, H)
    kernel.last_exec_time_ns = res.exec_time_ns
    kernel.last_results = res
    return out
